# revision 11
# baseline (speedup 1.0000x reference)
"""Fused Trainium2 kernel for nn_MultiHeadRelationalModule.

Data-parallel over 8 NeuronCores (8 samples each). The whole per-sample
pipeline (conv1 -> conv2 -> +coords -> K/Q/V proj -> LayerNorm ->
relational attention (4 heads, 596x596) -> softmax -> weighted sum ->
lin1 -> LN -> maxpool -> lin2 -> elu) runs on-chip; the big attention
maps never touch HBM.

Key identities used:
  elu(x) + 1 == max(x + 1, min(exp(x), 1))        (exact)
  A' = elu(z)+1 fed to matmul with alin_w: subtract colsum(alin_w) in the
       following bias to undo the +1 (softmax bias becomes
       alin_b - alin_w.sum(0)).
  softmax over c2 with A2^T layout (c2 on partitions): exp on chip,
       denominator via an appended ones-column on V in the E matmul.
  LN(x) = (x - mu) * rsqrt(var + eps); affine params in this model are
       identity (ones/zeros), verified at runtime.
  max-pool commutes with the final LN (monotone affine map).
"""

import numpy as np
from contextlib import ExitStack

import concourse.bacc as bacc
import concourse.bass as bass
import concourse.mybir as mybir
import concourse.tile as tile
from concourse.bass_utils import run_bass_kernel_spmd

F32 = mybir.dt.float32
AF = mybir.ActivationFunctionType
ALU = mybir.AluOpType

N_CORES = 8
SPB = 8               # samples per core
N_PIX = 596
HEADS = 4
D = 64
CH = [(0, 128), (128, 256), (256, 384), (384, 512), (512, 596)]
FH = [(0, 512), (512, 596)]
SHIFTS = [(0, 0), (0, 1), (1, 0), (1, 1)]
LN_N = float(HEADS * N_PIX * D)       # 152576
LN2_N = float(N_PIX * D)              # 38144
EPS = 1e-5

_cache = {}


def _prep_consts(inp):
    """Host-side preprocessing of weights into kernel-friendly layouts."""
    f = np.float32
    c = {}
    conv1_w = np.asarray(inp["conv1_w"], f)
    c["w1s"] = np.ascontiguousarray(
        np.concatenate([conv1_w[:, :, di, dj].T for (di, dj) in SHIFTS], axis=1)
    )  # (4, 64)
    c["b1"] = np.ascontiguousarray(np.asarray(inp["conv1_b"], f)[:, None])  # (16,1)
    conv2_w = np.asarray(inp["conv2_w"], f)
    c["w2s"] = np.ascontiguousarray(
        np.concatenate([conv2_w[:, :, di, dj].T for (di, dj) in SHIFTS], axis=1)
    )  # (16, 128)
    c["b2"] = np.ascontiguousarray(np.asarray(inp["conv2_b"], f)[:, None])  # (32,1)

    p = np.arange(N_PIX)
    c["coords"] = np.ascontiguousarray(
        np.stack([(p % 4) / 4.0, (p // 4) / 149.0]).astype(f)
    )  # (2, 596)

    c["kqvw"] = np.ascontiguousarray(
        np.concatenate(
            [np.asarray(inp["kp_w"], f), np.asarray(inp["qp_w"], f),
             np.asarray(inp["vp_w"], f)], axis=1)
    )  # (34, 768): K cols 0:256, Q 256:512, V 512:768

    qkb = np.zeros((64, 8), f)
    for h in range(HEADS):
        qkb[:, h] = np.asarray(inp["kp_b"], f)[h * 64:(h + 1) * 64]
        qkb[:, 4 + h] = np.asarray(inp["qp_b"], f)[h * 64:(h + 1) * 64]
    c["qkb"] = qkb

    vbb = np.zeros((128, 256), f)
    for h in range(HEADS):
        vbb[:, h * 64:(h + 1) * 64] = np.asarray(inp["vp_b"], f)[None,
                                                                 h * 64:(h + 1) * 64]
    c["vbb"] = vbb

    c["qklin"] = np.ascontiguousarray(
        np.concatenate([np.asarray(inp["qlin_w"], f),
                        np.asarray(inp["klin_w"], f)], axis=0)
    )  # (128, 596): rows 0:64 qlin (Q), 64:128 klin (K)

    qkbias = np.zeros((128, 10), f)
    qkl_b = np.asarray(inp["qlin_b"], f) + np.asarray(inp["klin_b"], f)
    for ci, (c0, c1) in enumerate(CH):
        qkbias[0:c1 - c0, ci] = qkl_b[c0:c1]
        qkbias[0:c1 - c0, 5 + ci] = qkl_b[c0:c1] + 1.0
    c["qkbias"] = qkbias

    c["alin"] = np.ascontiguousarray(np.asarray(inp["alin_w"], f))  # (596, 596)

    expb = np.zeros((128, 5), f)
    eb = np.asarray(inp["alin_b"], f) - np.asarray(inp["alin_w"], f).sum(axis=0)
    for ci, (c0, c1) in enumerate(CH):
        expb[0:c1 - c0, ci] = eb[c0:c1]
    c["expb"] = expb

    l1 = np.zeros((128, 128), f)
    lin1_w = np.asarray(inp["lin1_w"], f)
    l1[:, 0:64] = lin1_w[0:128]
    l1[:, 64:128] = lin1_w[128:256]
    c["lin1w"] = l1
    c["bl1"] = np.ascontiguousarray(np.asarray(inp["lin1_b"], f)[:, None])  # (64,1)
    c["lin2w"] = np.ascontiguousarray(np.asarray(inp["lin2_w"], f))  # (64,10)
    bl2 = np.zeros((10, 2), f)
    bl2[:, 0] = np.asarray(inp["lin2_b"], f)
    bl2[:, 1] = np.asarray(inp["lin2_b"], f) + 1.0
    c["bl2"] = bl2
    c["ones_r"] = np.ones((1, 128), f)
    c["ones_c"] = np.ones((128, 1), f)
    c["epsc"] = np.full((1, 1), EPS, f)
    return c


CONST_SHAPES = {
    "w1s": (4, 64), "b1": (16, 1), "w2s": (16, 128), "b2": (32, 1),
    "coords": (2, N_PIX), "kqvw": (34, 768), "qkb": (64, 8), "vbb": (128, 256),
    "qklin": (128, N_PIX), "qkbias": (128, 10), "alin": (N_PIX, N_PIX),
    "expb": (128, 5), "lin1w": (128, 128), "bl1": (64, 1), "lin2w": (64, 10),
    "bl2": (10, 2), "ones_r": (1, 128), "ones_c": (128, 1), "epsc": (1, 1),
}


def build_nc(spb=SPB):
    """Build the Bass program (same program runs SPMD on each core)."""
    nc = bacc.Bacc("TRN2", target_bir_lowering=False, debug=False)

    x_dram = nc.dram_tensor("x", [spb, 4, 151, 6], F32, kind="ExternalInput").ap()
    out_dram = nc.dram_tensor("out", [spb, 10], F32, kind="ExternalOutput").ap()
    cdram = {
        k: nc.dram_tensor(k, list(v), F32, kind="ExternalInput").ap()
        for k, v in CONST_SHAPES.items()
    }

    with tile.TileContext(nc) as tc, ExitStack() as ctx:
        pc = ctx.enter_context(tc.tile_pool(name="consts", bufs=1))
        # SBUF pools
        px = ctx.enter_context(tc.tile_pool(name="px", bufs=2))
        ph1 = ctx.enter_context(tc.tile_pool(name="ph1", bufs=2))
        pfeat = ctx.enter_context(tc.tile_pool(name="pfeat", bufs=2))
        pqk = ctx.enter_context(tc.tile_pool(name="pqk", bufs=8))
        pv = ctx.enter_context(tc.tile_pool(name="pv", bufs=40))
        pat = ctx.enter_context(tc.tile_pool(name="pat", bufs=7))
        pexp = ctx.enter_context(tc.tile_pool(name="pexp", bufs=3))
        psq = ctx.enter_context(tc.tile_pool(name="psq", bufs=2))
        pst = ctx.enter_context(tc.tile_pool(name="pst", bufs=3))
        peall = ctx.enter_context(tc.tile_pool(name="peall", bufs=4))
        pfix = ctx.enter_context(tc.tile_pool(name="pfix", bufs=1))
        # PSUM pools (8 banks total: 2+2+1+1+2)
        PS = bass.MemorySpace.PSUM
        ps_at = ctx.enter_context(tc.tile_pool(name="ps_at", bufs=2, space=PS))
        ps_a2 = ctx.enter_context(tc.tile_pool(name="ps_a2", bufs=2, space=PS))
        ps_e = ctx.enter_context(tc.tile_pool(name="ps_e", bufs=1, space=PS))
        ps_aux = ctx.enter_context(tc.tile_pool(name="ps_aux", bufs=1, space=PS))
        ps_main = ctx.enter_context(tc.tile_pool(name="ps_main", bufs=2, space=PS))

        # ---- load constants ----
        csb = {}
        for k, shp in CONST_SHAPES.items():
            if k == "alin":
                continue
            t = pc.tile(list(shp), F32, name=f"c_{k}")
            nc.sync.dma_start(out=t[:, :], in_=cdram[k][:, :])
            csb[k] = t
        alin_sb = []
        for ci, (c0, c1) in enumerate(CH):
            t = pc.tile([c1 - c0, N_PIX], F32, name=f"c_alin{ci}")
            nc.sync.dma_start(out=t[:, :], in_=cdram["alin"][c0:c1, :])
            alin_sb.append(t)

        emax_all = pfix.tile([64, spb], F32, name="emax_all")

        for s in range(spb):
            # ---------------- conv front-end ----------------
            x_t = px.tile([4, 151, 6], F32, name="x_t", tag="x")
            nc.sync.dma_start(out=x_t[:, :, :], in_=x_dram[s])

            h1 = ph1.tile([16, 750], F32, name="h1", tag="h1")
            h1v = h1.rearrange("c (h w) -> c h w", w=5)
            # conv1: two row-chunks (rows 0:102 -> 510 cols, 102:150 -> 240)
            for (r0, nr, dst0), pool in (((0, 102, 0), ps_main),
                                         ((102, 48, 510), ps_aux)):
                cps = pool.tile([16, nr * 5], F32, name="c1ps", tag="m512")
                for si, (di, dj) in enumerate(SHIFTS):
                    nc.tensor.matmul(
                        cps[:, :],
                        csb["w1s"][:, si * 16:(si + 1) * 16],
                        x_t[:, di + r0:di + r0 + nr, dj:dj + 5],
                        start=(si == 0), stop=(si == 3),
                    )
                nc.scalar.activation(h1[:, dst0:dst0 + nr * 5], cps[:, :],
                                     AF.Relu, bias=csb["b1"][:, 0:1])

            feats = pfeat.tile([34, N_PIX], F32, name="feats", tag="feats")
            nc.sync.dma_start(out=feats[32:34, :], in_=cdram["coords"][:, :])
            # conv2: rows 0:128 -> 512 cols, 128:149 -> 84
            for (r0, nr, dst0) in ((0, 128, 0), (128, 21, 512)):
                cps = ps_main.tile([32, nr * 4], F32, name="c2ps", tag="m512")
                for si, (di, dj) in enumerate(SHIFTS):
                    nc.tensor.matmul(
                        cps[:, :],
                        csb["w2s"][:, si * 32:(si + 1) * 32],
                        h1v[:, di + r0:di + r0 + nr, dj:dj + 4],
                        start=(si == 0), stop=(si == 3),
                    )
                nc.scalar.activation(feats[0:32, dst0:dst0 + nr * 4], cps[:, :],
                                     AF.Relu, bias=csb["b2"][:, 0:1])

            # ---------------- K/Q/V projections + LN stats ----------------
            # stats_qk cols: [Qs 0:8][Ks 8:16][Qsq 16:20][Ksq 20:24]
            stats_qk = pst.tile([64, 24], F32, name="stats_qk", tag="sqk")
            nc.vector.memset(stats_qk[:, :], 0.0)
            vstats = pst.tile([128, 40], F32, name="vstats", tag="vst")
            nc.vector.memset(vstats[:, :], 0.0)

            stacked = []
            sqs = psq.tile([64, N_PIX], F32, name="sqs", tag="sq")
            for h in range(HEADS):
                st_t = pqk.tile([128, N_PIX], F32, name="st_t", tag="qk")
                stacked.append(st_t)
                # Q -> rows 0:64 (matches qlin on top of qklin), K -> rows 64:128
                for (row0, off, bcol, scol0) in ((0, 256, 4 + h, 0),
                                                 (64, 0, h, 8)):
                    pps = ps_main.tile([64, 512], F32, name="pps", tag="m512")
                    pps2 = ps_main.tile([64, 84], F32, name="pps2", tag="m512")
                    nc.tensor.matmul(pps[:, :],
                                     csb["kqvw"][:, off + h * 64:off + h * 64 + 64],
                                     feats[:, 0:512], start=True, stop=True)
                    nc.tensor.matmul(pps2[:, :],
                                     csb["kqvw"][:, off + h * 64:off + h * 64 + 64],
                                     feats[:, 512:596], start=True, stop=True)
                    nc.scalar.activation(
                        st_t[row0:row0 + 64, 0:512], pps[:, :], AF.Identity,
                        bias=csb["qkb"][:, bcol:bcol + 1],
                        accum_out=stats_qk[:, scol0 + 2 * h:scol0 + 2 * h + 1])
                    nc.scalar.activation(
                        st_t[row0:row0 + 64, 512:596], pps2[:, :], AF.Identity,
                        bias=csb["qkb"][:, bcol:bcol + 1],
                        accum_out=stats_qk[:, scol0 + 2 * h + 1:scol0 + 2 * h + 2])
                # sum of squares over the full 596 (SBUF read)
                nc.scalar.activation(sqs[:, :], st_t[0:64, :], AF.Square,
                                     accum_out=stats_qk[:, 16 + h:17 + h])
                nc.scalar.activation(sqs[:, :], st_t[64:128, :], AF.Square,
                                     accum_out=stats_qk[:, 20 + h:21 + h])

            vtiles = []
            sqv = psq.tile([128, 64], F32, name="sqv", tag="sqv")
            for h in range(HEADS):
                vh = []
                for ci, (c0, c1) in enumerate(CH):
                    csz = c1 - c0
                    vps = ps_aux.tile([128, 64], F32, name="vps", tag="m512")
                    nc.tensor.matmul(vps[0:csz, :], feats[:, c0:c1],
                                     csb["kqvw"][:, 512 + h * 64:512 + h * 64 + 64],
                                     start=True, stop=True)
                    vt = pv.tile([128, 65], F32, name="vt", tag="v")
                    nc.vector.memset(vt[0:csz, 64:65], 1.0)
                    nc.vector.scalar_tensor_tensor(
                        vt[0:csz, 0:64], vps[0:csz, :], 1.0,
                        csb["vbb"][0:csz, h * 64:(h + 1) * 64],
                        op0=ALU.mult, op1=ALU.add,
                        accum_out=vstats[0:csz, h * 5 + ci:h * 5 + ci + 1])
                    nc.scalar.activation(sqv[0:csz, :], vt[0:csz, 0:64], AF.Square,
                                         accum_out=vstats[0:csz,
                                                          20 + h * 5 + ci:
                                                          21 + h * 5 + ci])
                    vh.append(vt)
                vtiles.append(vh)

            # ---------------- LN scalar pipeline ----------------
            qk2 = pst.tile([64, 4], F32, name="qk2", tag="qk2")
            qk2v = qk2.rearrange("p (a b) -> p a b", b=2)
            nc.vector.tensor_reduce(qk2v[:, :, 0:1],
                                    stats_qk[:, 0:16].rearrange(
                                        "p (a b) -> p a b", b=8),
                                    axis=mybir.AxisListType.X, op=ALU.add)
            nc.vector.tensor_reduce(qk2v[:, :, 1:2],
                                    stats_qk[:, 16:24].rearrange(
                                        "p (a b) -> p a b", b=4),
                                    axis=mybir.AxisListType.X, op=ALU.add)
            vred = pst.tile([128, 2], F32, name="vred", tag="vred")
            nc.vector.tensor_reduce(vred[:, :],
                                    vstats[:, :].rearrange(
                                        "p (a b) -> p a b", b=20),
                                    axis=mybir.AxisListType.X, op=ALU.add)
            stats_ps = ps_aux.tile([1, 6], F32, name="stats_ps", tag="m512")
            nc.tensor.matmul(stats_ps[0:1, 0:4], csb["ones_c"][0:64, 0:1],
                             qk2[:, :], start=True, stop=True)
            nc.tensor.matmul(stats_ps[0:1, 4:6], csb["ones_c"][0:128, 0:1],
                             vred[:, :], start=True, stop=True)
            # stats_ps = [sQ, ssqQ, sK, ssqK, sV, ssqV]
            spv = stats_ps.rearrange("p (a b) -> p a b", b=2)
            mu3 = pst.tile([1, 3], F32, name="mu3", tag="mu3")
            msq3 = pst.tile([1, 3], F32, name="msq3", tag="msq3")
            nc.vector.tensor_scalar_mul(mu3[:, :], spv[:, :, 0:1], 1.0 / LN_N)
            nc.vector.tensor_scalar_mul(msq3[:, :], spv[:, :, 1:2], 1.0 / LN_N)
            nmu2 = pst.tile([1, 3], F32, name="nmu2", tag="nmu2")
            nc.vector.scalar_tensor_tensor(nmu2[:, :], mu3[:, :], -1.0, mu3[:, :],
                                           op0=ALU.mult, op1=ALU.mult)
            var3 = pst.tile([1, 3], F32, name="var3", tag="var3")
            nc.vector.tensor_tensor(var3[:, :], msq3[:, :], nmu2[:, :], op=ALU.add)
            std3 = pst.tile([1, 3], F32, name="std3", tag="std3")
            nc.scalar.activation(std3[:, :], var3[:, :], AF.Sqrt,
                                 bias=csb["epsc"][0:1, 0:1])
            rsnmr = pst.tile([1, 6], F32, name="rsnmr", tag="rsnmr")
            rsv = rsnmr.rearrange("p (a b) -> p a b", b=2)
            nc.vector.reciprocal(rsv[:, :, 0:1], std3[:, :])
            nc.vector.scalar_tensor_tensor(rsv[:, :, 1:2], mu3[:, :], -1.0,
                                           rsv[:, :, 0:1],
                                           op0=ALU.mult, op1=ALU.mult)
            bc_ps = ps_aux.tile([128, 6], F32, name="bc_ps", tag="m512")
            nc.tensor.matmul(bc_ps[:, :], csb["ones_r"][0:1, :], rsnmr[:, :],
                             start=True, stop=True)
            bc = pst.tile([128, 6], F32, name="bc", tag="bc")
            nc.vector.tensor_copy(bc[:, :], bc_ps[:, :])
            # bc cols: [rsQ, nmrQ, rsK, nmrK, rsV, nmrV]

            # ---------------- LN apply (in place) ----------------
            for h in range(HEADS):
                nc.vector.tensor_scalar(stacked[h][0:64, :], stacked[h][0:64, :],
                                        bc[0:64, 0:1], bc[0:64, 1:2],
                                        op0=ALU.mult, op1=ALU.add)
                nc.vector.tensor_scalar(stacked[h][64:128, :], stacked[h][64:128, :],
                                        bc[0:64, 2:3], bc[0:64, 3:4],
                                        op0=ALU.mult, op1=ALU.add)
                for ci, (c0, c1) in enumerate(CH):
                    csz = c1 - c0
                    nc.vector.tensor_scalar(vtiles[h][ci][0:csz, 0:64],
                                            vtiles[h][ci][0:csz, 0:64],
                                            bc[0:csz, 4:5], bc[0:csz, 5:6],
                                            op0=ALU.mult, op1=ALU.add)

            # ---------------- attention ----------------
            eall = [peall.tile([128, N_PIX], F32, name=f"eall{i}", tag="eall")
                    for i in range(2)]
            for h in range(HEADS):
                for (f0, f1) in FH:
                    fsz = f1 - f0
                    at_tiles = []
                    for ci, (c0, c1) in enumerate(CH):
                        csz = c1 - c0
                        aps = ps_at.tile([128, 512], F32, name="aps", tag="at")
                        nc.tensor.matmul(aps[0:csz, 0:fsz],
                                         csb["qklin"][:, c0:c1],
                                         stacked[h][:, f0:f1],
                                         start=True, stop=True)
                        et = pexp.tile([128, 512], F32, name="et", tag="et")
                        nc.scalar.activation(et[0:csz, 0:fsz], aps[0:csz, 0:fsz],
                                             AF.Exp,
                                             bias=csb["qkbias"][0:csz, ci:ci + 1])
                        nc.gpsimd.tensor_scalar_min(et[0:csz, 0:fsz],
                                                    et[0:csz, 0:fsz], 1.0)
                        att = pat.tile([128, 512], F32, name="att", tag="atile")
                        nc.vector.scalar_tensor_tensor(
                            att[0:csz, 0:fsz], aps[0:csz, 0:fsz],
                            csb["qkbias"][0:csz, 5 + ci:6 + ci],
                            et[0:csz, 0:fsz], op0=ALU.add, op1=ALU.max)
                        at_tiles.append(att)
                    eps_t = ps_e.tile([65, 512], F32, name="eps_t", tag="e")
                    for c2i, (c20, c21) in enumerate(CH):
                        c2sz = c21 - c20
                        a2ps = ps_a2.tile([128, 512], F32, name="a2ps", tag="a2")
                        for ci, (c0, c1) in enumerate(CH):
                            csz = c1 - c0
                            nc.tensor.matmul(a2ps[0:c2sz, 0:fsz],
                                             alin_sb[ci][:, c20:c21],
                                             at_tiles[ci][0:csz, 0:fsz],
                                             start=(ci == 0), stop=(ci == 4))
                        ext = pexp.tile([128, 512], F32, name="ext", tag="et")
                        nc.scalar.activation(ext[0:c2sz, 0:fsz],
                                             a2ps[0:c2sz, 0:fsz], AF.Exp,
                                             bias=csb["expb"][0:c2sz, c2i:c2i + 1])
                        nc.tensor.matmul(eps_t[:, 0:fsz],
                                         vtiles[h][c2i][0:c2sz, 0:65],
                                         ext[0:c2sz, 0:fsz],
                                         start=(c2i == 0), stop=(c2i == 4))
                    recip = pst.tile([1, 512], F32, name="recip", tag="recip")
                    nc.vector.reciprocal(recip[0:1, 0:fsz], eps_t[64:65, 0:fsz])
                    bcp = ps_aux.tile([64, 512], F32, name="bcp", tag="m512")
                    nc.tensor.matmul(bcp[:, 0:fsz], csb["ones_r"][0:1, 0:64],
                                     recip[0:1, 0:fsz], start=True, stop=True)
                    bcs = pexp.tile([64, 512], F32, name="bcs", tag="bcs")
                    nc.scalar.copy(bcs[:, 0:fsz], bcp[:, 0:fsz])
                    nc.vector.tensor_tensor(
                        eall[h // 2][(h % 2) * 64:(h % 2) * 64 + 64, f0:f1],
                        eps_t[0:64, 0:fsz], bcs[:, 0:fsz], op=ALU.mult)

            # ---------------- lin1 + LN + max ----------------
            e2 = psq.tile([64, N_PIX], F32, name="e2", tag="sq")
            ls2 = pst.tile([64, 2], F32, name="ls2", tag="ls2")
            lpart = pst.tile([64, 2], F32, name="lpart", tag="lpart")
            for (f0, f1) in FH:
                fsz = f1 - f0
                lps = ps_main.tile([64, 512], F32, name="lps", tag="m512")
                for ck in range(2):
                    nc.tensor.matmul(lps[:, 0:fsz],
                                     csb["lin1w"][:, ck * 64:(ck + 1) * 64],
                                     eall[ck][:, f0:f1],
                                     start=(ck == 0), stop=(ck == 1))
                nc.scalar.activation(e2[:, f0:f1], lps[:, 0:fsz], AF.Relu,
                                     bias=csb["bl1"][:, 0:1],
                                     accum_out=lpart[:, (0 if f0 == 0 else 1):
                                                     (1 if f0 == 0 else 2)])
            nc.vector.tensor_reduce(ls2[:, 0:1], lpart[:, :],
                                    axis=mybir.AxisListType.X, op=ALU.add)
            nc.scalar.activation(sqs[:, :], e2[:, :], AF.Square,
                                 accum_out=ls2[:, 1:2])
            emaxv = pst.tile([64, 1], F32, name="emaxv", tag="emaxv")
            nc.vector.tensor_reduce(emaxv[:, :], e2[:, :],
                                    axis=mybir.AxisListType.X, op=ALU.max)
            st2 = ps_aux.tile([1, 2], F32, name="st2", tag="m512")
            nc.tensor.matmul(st2[0:1, :], csb["ones_c"][0:64, 0:1], ls2[:, :],
                             start=True, stop=True)
            mu2 = pst.tile([1, 2], F32, name="mu2", tag="mu2")
            nc.vector.tensor_scalar_mul(mu2[:, :], st2[:, :], 1.0 / LN2_N)
            nmu22 = pst.tile([1, 1], F32, name="nmu22", tag="nmu22")
            nc.vector.scalar_tensor_tensor(nmu22[:, :], mu2[:, 0:1], -1.0,
                                           mu2[:, 0:1], op0=ALU.mult, op1=ALU.mult)
            var2 = pst.tile([1, 1], F32, name="var2", tag="var2")
            nc.vector.tensor_tensor(var2[:, :], mu2[:, 1:2], nmu22[:, :],
                                    op=ALU.add)
            std2 = pst.tile([1, 1], F32, name="std2", tag="std2")
            nc.scalar.activation(std2[:, :], var2[:, :], AF.Sqrt,
                                 bias=csb["epsc"][0:1, 0:1])
            rsn2 = pst.tile([1, 2], F32, name="rsn2", tag="rsn2")
            nc.vector.reciprocal(rsn2[:, 0:1], std2[:, :])
            nc.vector.scalar_tensor_tensor(rsn2[:, 1:2], mu2[:, 0:1], -1.0,
                                           rsn2[:, 0:1], op0=ALU.mult, op1=ALU.mult)
            bc2p = ps_aux.tile([64, 2], F32, name="bc2p", tag="m512")
            nc.tensor.matmul(bc2p[:, :], csb["ones_r"][0:1, 0:64], rsn2[:, :],
                             start=True, stop=True)
            bc2 = pst.tile([64, 2], F32, name="bc2", tag="bc2")
            nc.vector.tensor_copy(bc2[:, :], bc2p[:, :])
            nc.vector.tensor_scalar(emax_all[:, s:s + 1], emaxv[:, :],
                                    bc2[:, 0:1], bc2[:, 1:2],
                                    op0=ALU.mult, op1=ALU.add)

        # ---------------- lin2 + final elu ----------------
        l2ps = ps_aux.tile([10, spb], F32, name="l2ps", tag="m512")
        nc.tensor.matmul(l2ps[:, :], csb["lin2w"][:, :], emax_all[:, :],
                         start=True, stop=True)
        fe = pst.tile([10, spb], F32, name="fe", tag="fe")
        nc.scalar.activation(fe[:, :], l2ps[:, :], AF.Exp,
                             bias=csb["bl2"][:, 0:1])
        nc.vector.tensor_scalar(fe[:, :], fe[:, :], 1.0, -1.0,
                                op0=ALU.min, op1=ALU.add)
        out_sb = pst.tile([10, spb], F32, name="out_sb", tag="out_sb")
        nc.vector.scalar_tensor_tensor(out_sb[:, :], l2ps[:, :],
                                       csb["bl2"][:, 0:1], fe[:, :],
                                       op0=ALU.add, op1=ALU.max)
        nc.sync.dma_start(out=out_dram.rearrange("s t -> t s"), in_=out_sb[:, :])

    return nc


def _reference_numpy(inp):
    """Pure-numpy fallback (only used if LN affine params are nontrivial)."""
    def ln(x, g=None, b=None):
        axes = tuple(range(1, x.ndim))
        mu = x.mean(axis=axes, keepdims=True)
        var = x.var(axis=axes, keepdims=True)
        y = (x - mu) / np.sqrt(var + EPS)
        return y * g + b if g is not None else y

    def elu(x):
        return np.where(x > 0, x, np.expm1(np.minimum(x, 0)))

    x = np.asarray(inp["x"], np.float64)
    N = x.shape[0]
    w1, b1 = np.asarray(inp["conv1_w"], np.float64), np.asarray(inp["conv1_b"], np.float64)
    h = np.zeros((N, 16, 150, 5))
    for di in range(2):
        for dj in range(2):
            h += np.einsum("oc,nchw->nohw", w1[:, :, di, dj],
                           x[:, :, di:di + 150, dj:dj + 5])
    h = np.maximum(h + b1[None, :, None, None], 0)
    w2, b2 = np.asarray(inp["conv2_w"], np.float64), np.asarray(inp["conv2_b"], np.float64)
    h2 = np.zeros((N, 32, 149, 4))
    for di in range(2):
        for dj in range(2):
            h2 += np.einsum("oc,nchw->nohw", w2[:, :, di, dj],
                            h[:, :, di:di + 149, dj:dj + 4])
    h2 = np.maximum(h2 + b2[None, :, None, None], 0)
    p = np.arange(N_PIX)
    xc, yc = (p % 4) / 4.0, (p // 4) / 149.0
    feats = np.concatenate(
        [h2.transpose(0, 2, 3, 1).reshape(N, N_PIX, 32),
         np.broadcast_to(np.stack([xc, yc], 1)[None], (N, N_PIX, 2))], axis=2)

    def proj(wn, bn, gn, bn2):
        P = (feats @ np.asarray(inp[wn], np.float64) + np.asarray(inp[bn], np.float64))
        P = P.reshape(N, N_PIX, HEADS, D).transpose(0, 2, 1, 3)
        return ln(P, np.asarray(inp[gn], np.float64), np.asarray(inp[bn2], np.float64))

    K = proj("kp_w", "kp_b", "knorm_g", "knorm_b")
    Q = proj("qp_w", "qp_b", "qnorm_g", "qnorm_b")
    V = proj("vp_w", "vp_b", "vnorm_g", "vnorm_b")
    A = elu(Q @ np.asarray(inp["qlin_w"], np.float64) + np.asarray(inp["qlin_b"], np.float64)
            + K @ np.asarray(inp["klin_w"], np.float64) + np.asarray(inp["klin_b"], np.float64))
    A = A @ np.asarray(inp["alin_w"], np.float64) + np.asarray(inp["alin_b"], np.float64)
    A = A - A.max(axis=-1, keepdims=True)
    A = np.exp(A)
    A = A / A.sum(axis=-1, keepdims=True)
    E = np.einsum("bhfc,bhcd->bhfd", A, V)
    E = E.transpose(0, 2, 1, 3).reshape(N, N_PIX, HEADS * D)
    E = np.maximum(E @ np.asarray(inp["lin1_w"], np.float64)
                   + np.asarray(inp["lin1_b"], np.float64), 0)
    E = ln(E)
    E = E.max(axis=1)
    out = E @ np.asarray(inp["lin2_w"], np.float64) + np.asarray(inp["lin2_b"], np.float64)
    return elu(out).astype(np.float32)


def kernel(**inputs):
    trivial = (np.all(np.asarray(inputs["knorm_g"]) == 1.0)
               and np.all(np.asarray(inputs["knorm_b"]) == 0.0)
               and np.all(np.asarray(inputs["qnorm_g"]) == 1.0)
               and np.all(np.asarray(inputs["qnorm_b"]) == 0.0)
               and np.all(np.asarray(inputs["vnorm_g"]) == 1.0)
               and np.all(np.asarray(inputs["vnorm_b"]) == 0.0))
    if not trivial:
        return _reference_numpy(inputs)

    x = np.ascontiguousarray(np.asarray(inputs["x"], np.float32))
    n = x.shape[0]
    assert n == N_CORES * SPB, f"expected batch {N_CORES * SPB}, got {n}"
    consts = _prep_consts(inputs)

    if "nc" not in _cache:
        nc = build_nc(SPB)
        nc.compile()
        _cache["nc"] = nc
    nc = _cache["nc"]

    in_maps = []
    for c in range(N_CORES):
        m = dict(consts)
        m["x"] = np.ascontiguousarray(x[c * SPB:(c + 1) * SPB])
        in_maps.append(m)

    import os
    trace = bool(int(os.environ.get("KERNEL_TRACE", "0")))
    res = run_bass_kernel_spmd(nc, in_maps, list(range(N_CORES)), trace=trace)
    kernel._last_results = res
    out = np.concatenate([np.asarray(r["out"]) for r in res.results], axis=0)
    return out.astype(np.float32)


kernel._last_results = None

